# revision 18
# baseline (speedup 1.0000x reference)
"""GatedDeltaNet on Trainium2, 8 NeuronCores (SPMD).

Sharding: 8 cores = 2 batch groups x 4 cores. Within a batch group each core
owns one full head (512 v-cols) + one half head (256 v-cols) of the value/gate
path (768 of 3072 hv columns), and computes q/k projections for its 2 heads.
The T=4096 recurrence uses the chunked WY (gated delta rule) formulation,
C=128 chunks: per chunk a strictly-lower M is built from decay-weighted K K^T,
T=(I+M)^{-1} via the nilpotent squaring identity (7 factors), then U = T R,
O = exp(g)Q S0 + A U, S' = exp(gC) S0 + K~^T U.  All matmuls bf16 / fp32
PSUM; q/k l2-norms and beta fold into log-domain exponent tiles.  Cross-core:
pairwise AllReduce of split-head RMSNorm sum-of-squares + per-block
ReduceScatter of Wo partial sums.

kernel(**inputs) takes FULL inputs, returns FULL [B,T,HID] fp32 output.
"""
import os
from contextlib import ExitStack
import numpy as np
import ml_dtypes

H, DK, DV = 6, 256, 512
HID = 2048
CONV = 4
NORM_EPS = 1e-5
N_CORES = 8
T = 4096
C = 128            # chunk length
NCH = T // C       # 32 chunks
NB = 8             # proj token blocks of 512
NOB = 4            # output blocks of 1024 tokens (8 chunks each)
QKF = 512          # q/k feature cols per core (2 heads)
HVF = 768          # v/gate cols per core (full head 512 + half head 256)

FULL_HEAD = [0, 2, 3, 5]
HALF_HEAD = [1, 1, 4, 4]
HALF_LO = [True, False, True, False]

_DEBUG = os.environ.get("GDN_DEBUG", "") == "1"


def _build(nc):
    import concourse.bass as bass  # noqa
    import concourse.mybir as mybir
    import concourse.tile as tile

    bf16 = mybir.dt.bfloat16
    f32 = mybir.dt.float32

    P = nc.declare_dram_parameter
    io = dict(
        xT=P("xT", [HID, T], bf16, isOutput=False),
        wqT=P("wqT", [HID, QKF], bf16, isOutput=False),
        wkT=P("wkT", [HID, QKF], bf16, isOutput=False),
        wabT=P("wabT", [HID, 64], bf16, isOutput=False),
        wvT=P("wvT", [HID, HVF], bf16, isOutput=False),
        wgT=P("wgT", [HID, HVF], bf16, isOutput=False),
        woT=P("woT", [HVF, HID], bf16, isOutput=False),
        convw=P("convw", [QKF * 2 + HVF, CONV], f32, isOutput=False),
        dtb=P("dtb", [2, 1], f32, isOutput=False),
        negA=P("negA", [2, 1], f32, isOutput=False),
        id_bf=P("id_bf", [128, 128], bf16, isOutput=False),
        id_f32=P("id_f32", [128, 128], f32, isOutput=False),
        ones_bf=P("ones_bf", [128, 128], bf16, isOutput=False),
        # masks: [:,0:128] strictly-lower=-1 ; [:,128:256] strictly-upper=-1 ;
        #        [:,256:384] upper-incl-diag=+1
        masks=P("masks", [128, 384], f32, isOutput=False),
        out=P("out", [NOB * 256, HID], bf16, isOutput=True),
    )
    dbg = {}
    if _DEBUG:
        dbg["kt0"] = P("dbg_kt0", [128, 256], bf16, isOutput=True)
        dbg["gamT"] = P("dbg_gamT", [128, 64], f32, isOutput=True)
        dbg["sskT"] = P("dbg_sskT", [128, 64], f32, isOutput=True)
        dbg["c1"] = P("dbg_c1", [128, 64], f32, isOutput=True)
        dbg["N0"] = P("dbg_N0", [128, 128], bf16, isOutput=True)
        dbg["Tt0"] = P("dbg_Tt0", [128, 128], bf16, isOutput=True)
        dbg["o01"] = P("dbg_o01", [128, 1536], bf16, isOutput=True)
        dbg["U0"] = P("dbg_U0", [128, 512], bf16, isOutput=True)
        dbg["gs"] = P("dbg_gs", [64, 128], f32, isOutput=True)
        dbg["gams"] = P("dbg_gams", [64, 128], f32, isOutput=True)
        dbg["abg"] = P("dbg_abg", [2, 512], f32, isOutput=True)
        dbg["kt2"] = P("dbg_kt2", [128, 256], bf16, isOutput=True)
        dbg["ssqT"] = P("dbg_ssqT", [128, 64], f32, isOutput=True)
    io["dbg"] = dbg

    with tile.TileContext(nc) as tc:
        _emit(nc, tc, mybir, io)
    return nc


def _emit(nc, tc, mybir, io):
    bf16 = mybir.dt.bfloat16
    f32 = mybir.dt.float32
    AF = mybir.ActivationFunctionType
    OP = mybir.AluOpType
    dbg = io["dbg"]
    out = io["out"]

    es = ExitStack()
    const = es.enter_context(tc.tile_pool(name="const", bufs=1))
    dram = es.enter_context(tc.tile_pool(name="dram", bufs=1, space="DRAM"))
    persist = es.enter_context(tc.tile_pool(name="persist", bufs=1))

    id_bf_sb = const.tile([128, 128], bf16, tag="idb")
    nc.sync.dma_start(id_bf_sb[:], io["id_bf"][:])
    id_f32_sb = const.tile([128, 128], f32, tag="idf")
    nc.sync.dma_start(id_f32_sb[:], io["id_f32"][:])
    ones_sb = const.tile([128, 128], bf16, tag="ones")
    nc.sync.dma_start(ones_sb[:], io["ones_bf"][:])
    masks_sb = const.tile([128, 384], f32, tag="masks")
    nc.sync.dma_start(masks_sb[:], io["masks"][:])
    convw_sb = const.tile([128, 14 * CONV], f32, tag="cw")
    nc.sync.dma_start(convw_sb[:].rearrange("p (n k) -> p n k", k=CONV),
                      io["convw"][:].rearrange("(n p) k -> p n k", p=128))
    dtb_sb = const.tile([2, 1], f32, tag="dtb")
    nc.sync.dma_start(dtb_sb[:], io["dtb"][:])
    negA_sb = const.tile([2, 1], f32, tag="negA")
    nc.sync.dma_start(negA_sb[:], io["negA"][:])
    ones_f32 = const.tile([1, 128], f32, tag="onesf")
    nc.vector.memset(ones_f32[:], 1.0)
    eps6_sb = const.tile([128, 1], f32, tag="eps6")
    nc.vector.memset(eps6_sb[:], 1e-6)
    epsN_sb = const.tile([128, 1], f32, tag="epsN")
    nc.vector.memset(epsN_sb[:], NORM_EPS)

    qraw_d = dram.tile([QKF, T], bf16)
    kraw_d = dram.tile([QKF, T], bf16)
    vraw_d = dram.tile([HVF, T], bf16)
    vclean_d = dram.tile([HVF, T], bf16)
    gate_d = dram.tile([T, HVF], bf16)
    ab_d = dram.tile([6, T], f32)
    var_in_d = [dram.tile([128, 8], f32, name=f"var_in{b}") for b in range(NOB)]
    var_out_d = [dram.tile([128, 8], f32, name=f"var_out{b}") for b in range(NOB)]
    rs_in_d = [dram.tile([1024, HID], bf16, name=f"rs_in{b}") for b in range(NOB)]
    rs_out_d = [dram.tile([256, HID], bf16, name=f"rs_out{b}") for b in range(NOB)]

    ab_sb = persist.tile([64, T], f32, tag="ab")

    # ============ PHASE 1: projections (stream xT blocks) ============
    with tc.tile_pool(name="wts", bufs=1) as wts, \
         tc.tile_pool(name="xtp", bufs=2) as xtp, \
         tc.tile_pool(name="pjps", bufs=6, space="PSUM") as pjps, \
         tc.tile_pool(name="pjsb", bufs=4) as pjsb:
        wq_sb = wts.tile([128, 16 * QKF], bf16, tag="wq")
        nc.sync.dma_start(wq_sb[:].rearrange("p (n m) -> p n m", n=16),
                          io["wqT"][:].rearrange("(n p) m -> p n m", p=128))
        wk_sb = wts.tile([128, 16 * QKF], bf16, tag="wk")
        nc.sync.dma_start(wk_sb[:].rearrange("p (n m) -> p n m", n=16),
                          io["wkT"][:].rearrange("(n p) m -> p n m", p=128))
        wab_sb = wts.tile([128, 16 * 64], bf16, tag="wab")
        nc.sync.dma_start(wab_sb[:].rearrange("p (n m) -> p n m", n=16),
                          io["wabT"][:].rearrange("(n p) m -> p n m", p=128))
        wv_sb = wts.tile([128, 16 * HVF], bf16, tag="wv")
        nc.sync.dma_start(wv_sb[:].rearrange("p (n m) -> p n m", n=16),
                          io["wvT"][:].rearrange("(n p) m -> p n m", p=128))
        wg_sb = wts.tile([128, 16 * HVF], bf16, tag="wg")
        nc.sync.dma_start(wg_sb[:].rearrange("p (n m) -> p n m", n=16),
                          io["wgT"][:].rearrange("(n p) m -> p n m", p=128))

        for j in range(NB):
            xt_b = xtp.tile([128, 16 * 512], bf16, tag="xt")
            nc.sync.dma_start(
                xt_b[:].rearrange("p (n t) -> p n t", n=16),
                io["xT"][:, j * 512:(j + 1) * 512]
                .rearrange("(n p) t -> p n t", p=128))

            def proj_p2(w_sb, mtiles, wtot, dst_d):
                for m in range(mtiles):
                    ps = pjps.tile([128, 512], f32, tag="pp")
                    for k in range(16):
                        nc.tensor.matmul(
                            ps[:], lhsT=w_sb[:, k * wtot + m * 128: k * wtot + (m + 1) * 128],
                            rhs=xt_b[:, k * 512:(k + 1) * 512],
                            start=(k == 0), stop=(k == 15))
                    sc = pjsb.tile([128, 512], bf16, tag="ev")
                    nc.scalar.copy(sc[:], ps[:])
                    nc.sync.dma_start(
                        dst_d[m * 128:(m + 1) * 128, j * 512:(j + 1) * 512], sc[:])

            proj_p2(wq_sb, 4, QKF, qraw_d)
            proj_p2(wk_sb, 4, QKF, kraw_d)
            proj_p2(wv_sb, 6, HVF, vraw_d)
            # ab proj (a rows at partitions 0-1, b rows at 32-33)
            ps_ab = pjps.tile([128, 512], f32, tag="pp")
            for k in range(16):
                nc.tensor.matmul(ps_ab[0:64, :], lhsT=wab_sb[:, k * 64:(k + 1) * 64],
                                 rhs=xt_b[:, k * 512:(k + 1) * 512],
                                 start=(k == 0), stop=(k == 15))
            nc.vector.tensor_copy(ab_sb[:, j * 512:(j + 1) * 512], ps_ab[0:64, :])
            # gate: P1 token-major [tok, 768]
            for s in range(4):
                for f in range(2):
                    fn = 512 if f == 0 else 256
                    ps = pjps.tile([128, 512], f32, tag="pp")
                    for k in range(16):
                        nc.tensor.matmul(
                            ps[:, 0:fn],
                            lhsT=xt_b[:, k * 512 + s * 128: k * 512 + (s + 1) * 128],
                            rhs=wg_sb[:, k * HVF + f * 512: k * HVF + f * 512 + fn],
                            start=(k == 0), stop=(k == 15))
                    sc = pjsb.tile([128, 512], bf16, tag="ev")
                    nc.scalar.copy(sc[:, 0:fn], ps[:, 0:fn])
                    nc.sync.dma_start(
                        gate_d[j * 512 + s * 128: j * 512 + (s + 1) * 128,
                               f * 512: f * 512 + fn], sc[:, 0:fn])

    # kt/qt allocated only after proj phase frees its pools
    ktqt = es.enter_context(tc.tile_pool(name="ktqt", bufs=1))
    kt = [ktqt.tile([128, T], bf16, tag=f"kt{i}", name=f"kt{i}") for i in range(4)]
    qt = [ktqt.tile([128, T], bf16, tag=f"qt{i}", name=f"qt{i}") for i in range(4)]

    # ============ PHASE 2: conv + silu for q/k/v ============
    with tc.tile_pool(name="cvin", bufs=2) as cvin, \
         tc.tile_pool(name="cvy", bufs=2) as cvy:
        def conv_silu(src_d, ftile_global, row, dst_sb=None, dst_d=None):
            raw = cvin.tile([128, T], bf16, tag="raw")
            nc.sync.dma_start(raw[:], src_d[row * 128:(row + 1) * 128, :])
            y = cvy.tile([128, T], f32, tag="y")
            w = lambda j: convw_sb[:, ftile_global * CONV + j: ftile_global * CONV + j + 1]
            nc.scalar.activation(y[:], raw[:], AF.Copy, bias=0.0, scale=w(3))
            for s in range(1, CONV):
                nc.vector.scalar_tensor_tensor(
                    y[:, s:T], in0=raw[:, 0:T - s], scalar=w(3 - s), in1=y[:, s:T],
                    op0=OP.mult, op1=OP.add)
            if dst_sb is not None:
                nc.scalar.activation(dst_sb[:], y[:], AF.Silu)
            else:
                o = cvin.tile([128, T], bf16, tag="vcl")
                nc.scalar.activation(o[:], y[:], AF.Silu)
                nc.sync.dma_start(dst_d[row * 128:(row + 1) * 128, :], o[:])

        for m in range(4):
            conv_silu(qraw_d, m, m, dst_sb=qt[m])
        for m in range(4):
            conv_silu(kraw_d, 4 + m, m, dst_sb=kt[m])
        for m in range(6):
            conv_silu(vraw_d, 8 + m, m, dst_d=vclean_d)

    # ============ PHASE 3: g / beta / log-domain prep ============
    with tc.tile_pool(name="abp", bufs=1) as abp:
        # softplus(x+dtb) = ln(1+exp(x+dtb)); sigmoid/log-sigmoid via exp
        g2 = abp.tile([2, T], f32, tag="row2")
        nc.scalar.activation(g2[:], ab_sb[0:2, :], AF.Exp, bias=dtb_sb[:, 0:1])
        nc.scalar.activation(g2[:], g2[:], AF.Ln, bias=1.0)
        nc.vector.tensor_scalar_mul(g2[:], g2[:], negA_sb[:, 0:1])
        nc.sync.dma_start(ab_d[0:2, :], g2[:])
        eneg = abp.tile([2, T], f32, tag="row2b")
        nc.scalar.activation(eneg[:], ab_sb[32:34, :], AF.Exp, scale=-1.0)
        lnb2 = abp.tile([2, T], f32, tag="row2c")
        nc.scalar.activation(lnb2[:], eneg[:], AF.Ln, bias=1.0)
        beta2 = abp.tile([2, T], f32, tag="row2d")
        nc.scalar.activation(beta2[:], lnb2[:], AF.Exp, scale=-1.0)
        nc.sync.dma_start(ab_d[2:4, :], beta2[:])
        nc.vector.tensor_scalar_mul(lnb2[:], lnb2[:], -1.0)
        nc.sync.dma_start(ab_d[4:6, :], lnb2[:])

    gs = persist.tile([64, 128], f32, tag="gs")
    bts = persist.tile([64, 128], f32, tag="bts")
    lbs = persist.tile([64, 128], f32, tag="lbs")
    for hh in range(2):
        nc.sync.dma_start(gs[hh * 32:(hh + 1) * 32, :],
                          ab_d[hh:hh + 1, :].rearrange("o (c t) -> (o c) t", c=NCH))
        nc.sync.dma_start(bts[hh * 32:(hh + 1) * 32, :],
                          ab_d[2 + hh:3 + hh, :].rearrange("o (c t) -> (o c) t", c=NCH))
        nc.sync.dma_start(lbs[hh * 32:(hh + 1) * 32, :],
                          ab_d[4 + hh:5 + hh, :].rearrange("o (c t) -> (o c) t", c=NCH))
    zer64 = persist.tile([64, 128], f32, tag="z64")
    nc.vector.memset(zer64[:], 0.0)
    gams = persist.tile([64, 128], f32, tag="gams")
    nc.vector.tensor_tensor_scan(gams[:], gs[:], zer64[:], 0.0, op0=OP.add, op1=OP.add)

    with tc.tile_pool(name="prps", bufs=2, space="PSUM") as prps, \
         tc.tile_pool(name="sqp", bufs=1) as sqp:
        def transpose_f32(src, tag):
            ps = prps.tile([128, 128], f32, tag="pr")
            nc.tensor.transpose(ps[0:128, 0:64], src[:], id_f32_sb[0:64, 0:64])
            d = persist.tile([128, 64], f32, tag=tag)
            nc.vector.tensor_copy(d[:], ps[0:128, 0:64])
            return d

        gamT = transpose_f32(gams, "gamT")
        lnbT = transpose_f32(lbs, "lnbT")
        betT = transpose_f32(bts, "betT")

        # batched per-(chunk,head) sum-of-squares of k and q
        sskT = persist.tile([128, 64], f32, tag="sskT")
        ssqT = persist.tile([128, 64], f32, tag="ssqT")
        for hi in range(2):
            sq = [sqp.tile([128, T], bf16, tag=f"sq{i}", name=f"sq{i}") for i in range(4)]
            nc.scalar.activation(sq[0][:], kt[hi * 2][:], AF.Square)
            nc.scalar.activation(sq[1][:], kt[hi * 2 + 1][:], AF.Square)
            nc.scalar.activation(sq[2][:], qt[hi * 2][:], AF.Square)
            nc.scalar.activation(sq[3][:], qt[hi * 2 + 1][:], AF.Square)
            for c in range(NCH):
                j = hi * NCH + c
                ps = prps.tile([128, 1], f32, tag="ss")
                nc.tensor.matmul(ps[:], lhsT=sq[0][:, c * C:(c + 1) * C],
                                 rhs=ones_sb[:, 0:1], start=True, stop=False)
                nc.tensor.matmul(ps[:], lhsT=sq[1][:, c * C:(c + 1) * C],
                                 rhs=ones_sb[:, 0:1], start=False, stop=True)
                nc.vector.tensor_copy(sskT[:, j:j + 1], ps[:])
                ps2 = prps.tile([128, 1], f32, tag="ss")
                nc.tensor.matmul(ps2[:], lhsT=sq[2][:, c * C:(c + 1) * C],
                                 rhs=ones_sb[:, 0:1], start=True, stop=False)
                nc.tensor.matmul(ps2[:], lhsT=sq[3][:, c * C:(c + 1) * C],
                                 rhs=ones_sb[:, 0:1], start=False, stop=True)
                nc.vector.tensor_copy(ssqT[:, j:j + 1], ps2[:])

        # batched log-domain combos [128, 64]
        Lk = persist.tile([128, 64], f32, tag="Lk")
        nc.scalar.activation(Lk[:], sskT[:], AF.Ln, bias=eps6_sb[:, 0:1])
        Lq = persist.tile([128, 64], f32, tag="Lq")
        nc.scalar.activation(Lq[:], ssqT[:], AF.Ln, bias=eps6_sb[:, 0:1])
        c1 = persist.tile([128, 64], f32, tag="c1")
        nc.vector.scalar_tensor_tensor(c1[:], in0=Lk[:], scalar=-0.5, in1=gamT[:],
                                       op0=OP.mult, op1=OP.add)
        nc.vector.tensor_tensor(c1[:], c1[:], lnbT[:], OP.add)
        c3 = persist.tile([128, 64], f32, tag="c3")
        nc.vector.scalar_tensor_tensor(c3[:], in0=Lq[:], scalar=-0.5, in1=gamT[:],
                                       op0=OP.mult, op1=OP.add)
        nc.vector.tensor_scalar_add(c3[:], c3[:], float(np.log(DK ** -0.5)))
        c4 = persist.tile([128, 64], f32, tag="c4")
        nc.vector.scalar_tensor_tensor(c4[:], in0=Lk[:], scalar=-0.5, in1=gamT[:],
                                       op0=OP.mult, op1=OP.subtract)
        # gamC broadcast [128, 64]
        ps_r = prps.tile([128, 128], f32, tag="pr")
        nc.tensor.matmul(ps_r[0:1, 0:64], lhsT=gams[:, 127:128], rhs=id_f32_sb[0:64, 0:64],
                         start=True, stop=True)
        gcrow = persist.tile([1, 64], f32, tag="gcrow")
        nc.vector.tensor_copy(gcrow[:], ps_r[0:1, 0:64])
        ps_b = prps.tile([128, 128], f32, tag="pr")
        nc.tensor.matmul(ps_b[0:128, 0:64], lhsT=ones_f32[0:1, 0:128], rhs=gcrow[:],
                         start=True, stop=True)
        gcb = persist.tile([128, 64], f32, tag="gcb")
        nc.vector.tensor_copy(gcb[:], ps_b[0:128, 0:64])
        exp_gc = persist.tile([128, 64], f32, tag="egc")
        nc.scalar.activation(exp_gc[:], gcb[:], AF.Exp)
        c5 = persist.tile([128, 64], f32, tag="c5")
        nc.vector.tensor_tensor(c5[:], c4[:], gcb[:], OP.add)
        exp_st = persist.tile([128, 64], f32, tag="est")
        nc.scalar.activation(exp_st[:], c5[:], AF.Exp)
        nexp_R = persist.tile([128, 64], f32, tag="nexpR")
        nc.scalar.activation(nexp_R[:], c1[:], AF.Exp)
        nc.vector.tensor_scalar_mul(nexp_R[:], nexp_R[:], -1.0)
        exp_O = persist.tile([128, 64], f32, tag="expO")
        nc.scalar.activation(exp_O[:], c3[:], AF.Exp)

        def col2rows(csrc, tag):
            # transpose [128,64] -> [64,128] in f32 (log-domain values are
            # too large for bf16), then DRAM-bounce so each row is a [1,128]
            # AP at base partition 0 (matmul requires matching bases)
            ps = prps.tile([64, 128], f32, tag="tpr")
            nc.tensor.transpose(ps[:], csrc[:], id_f32_sb[:])
            rtmp = persist.tile([64, 128], f32, tag=tag + "t")
            nc.scalar.copy(rtmp[:], ps[:])
            rd = dram.tile([64, 128], f32, name="rows_" + tag)
            nc.sync.dma_start(rd[:], rtmp[:])
            return rd
        r1 = col2rows(c4, "r1")
        r2 = col2rows(c1, "r2")
        r3 = col2rows(c3, "r3")

    if _DEBUG:
        nc.sync.dma_start(dbg["gs"][:], gs[:])
        nc.sync.dma_start(dbg["gams"][:], gams[:])
        nc.sync.dma_start(dbg["abg"][:], ab_d[0:2, 0:512])
        nc.sync.dma_start(dbg["kt2"][:, 0:128], kt[2][:, 0:128])
        nc.sync.dma_start(dbg["kt2"][:, 128:256], kt[3][:, 0:128])
        nc.sync.dma_start(dbg["ssqT"][:], ssqT[:])
        nc.sync.dma_start(dbg["kt0"][:, 0:128], kt[0][:, 0:128])
        nc.sync.dma_start(dbg["kt0"][:, 128:256], kt[1][:, 0:128])
        nc.sync.dma_start(dbg["gamT"][:], gamT[:])
        nc.sync.dma_start(dbg["sskT"][:], sskT[:])
        nc.sync.dma_start(dbg["c1"][:], c1[:])

    # ============ PHASE 4/5: recurrence + output ============
    o_d = dram.tile([T, HVF], bf16, name="o_d")
    ssqA = persist.tile([128, NCH], f32, tag="ssqA")
    ssqB = persist.tile([128, NCH], f32, tag="ssqB")
    S_A = [persist.tile([128, 512], bf16, tag=f"SA{d}", name=f"SA{d}") for d in range(2)]
    S_B = [persist.tile([128, 256], bf16, tag=f"SB{d}", name=f"SB{d}") for d in range(2)]
    for d in range(2):
        nc.vector.memset(S_A[d][:], 0.0)
        nc.vector.memset(S_B[d][:], 0.0)

    wop = es.enter_context(tc.tile_pool(name="wop", bufs=1))
    wo_sb = wop.tile([128, 6 * HID], bf16, tag="wo")
    nc.sync.dma_start(wo_sb[:].rearrange("p (n m) -> p n m", n=6),
                      io["woT"][:].rearrange("(n p) m -> p n m", p=128))

    psS = es.enter_context(tc.tile_pool(name="psS", bufs=2, space="PSUM"))
    psB = es.enter_context(tc.tile_pool(name="psB", bufs=3, space="PSUM"))
    wk_p = es.enter_context(tc.tile_pool(name="wkp", bufs=2))

    MLOW = masks_sb[:, 0:128]
    MUPP = masks_sb[:, 128:256]
    MUPI = masks_sb[:, 256:384]

    def exp_tile(row_src, bias_col, tag):
        row = wk_p.tile([1, 128], f32, tag="row")
        nc.sync.dma_start(row[:], row_src)
        ps = psS.tile([128, 128], f32, tag="sf")
        nc.tensor.matmul(ps[:], lhsT=ones_f32[0:1, 0:128], rhs=row[:], start=True, stop=True)
        arg = wk_p.tile([128, 128], f32, tag="arg")
        nc.vector.tensor_scalar(arg[:], ps[:], bias_col, 20.0, op0=OP.add, op1=OP.min)
        e = wk_p.tile([128, 128], bf16, tag="e" + tag)
        nc.scalar.activation(e[:], arg[:], AF.Exp)
        return e

    def chunk_head(c, hi, ktA, ktB, qtA, qtB, S, DVh, o_off, vrow0, nvt, ssq_dst):
        j = hi * NCH + c
        sl = slice(c * C, (c + 1) * C)
        eM = exp_tile(r1[j:j + 1, :], c1[:, j:j + 1], "M")
        eNt = exp_tile(r2[j:j + 1, :], c4[:, j:j + 1], "Nt")
        eAt = exp_tile(r3[j:j + 1, :], c4[:, j:j + 1], "At")
        ps_kk = psS.tile([128, 128], f32, tag="sf")
        nc.tensor.matmul(ps_kk[:], lhsT=ktA[:, sl], rhs=ktA[:, sl], start=True, stop=False)
        nc.tensor.matmul(ps_kk[:], lhsT=ktB[:, sl], rhs=ktB[:, sl], start=False, stop=True)
        kkl = wk_p.tile([128, 128], bf16, tag="kkl")
        nc.vector.tensor_tensor(kkl[:], ps_kk[:], MLOW, OP.mult)
        kku = wk_p.tile([128, 128], bf16, tag="kku")
        nc.vector.tensor_tensor(kku[:], ps_kk[:], MUPP, OP.mult)
        ps_kq = psS.tile([128, 128], f32, tag="sf")
        nc.tensor.matmul(ps_kq[:], lhsT=ktA[:, sl], rhs=qtA[:, sl], start=True, stop=False)
        nc.tensor.matmul(ps_kq[:], lhsT=ktB[:, sl], rhs=qtB[:, sl], start=False, stop=True)
        kqu = wk_p.tile([128, 128], bf16, tag="kqu")
        nc.vector.tensor_tensor(kqu[:], ps_kq[:], MUPI, OP.mult)
        N = wk_p.tile([128, 128], bf16, tag="N")
        nc.vector.tensor_tensor(N[:], eM[:], kkl[:], OP.mult)
        Nt = wk_p.tile([128, 128], bf16, tag="Nt")
        nc.vector.tensor_tensor(Nt[:], eNt[:], kku[:], OP.mult)
        At = wk_p.tile([128, 128], bf16, tag="At")
        nc.vector.tensor_tensor(At[:], eAt[:], kqu[:], OP.mult)
        # T build
        Tt = wk_p.tile([128, 128], bf16, tag="Tt")
        nc.vector.tensor_tensor(Tt[:], Nt[:], id_bf_sb[:], OP.add)
        Na, Nta = N, Nt
        for i in range(1, 7):
            ps = psS.tile([128, 128], f32, tag="sf")
            nc.tensor.matmul(ps[:], lhsT=Nta[:], rhs=Na[:], start=True, stop=True)
            N2 = wk_p.tile([128, 128], bf16, tag="N2")
            nc.scalar.copy(N2[:], ps[:])
            if i < 6:
                ps2 = psS.tile([128, 128], f32, tag="sf")
                nc.tensor.matmul(ps2[:], lhsT=Na[:], rhs=Nta[:], start=True, stop=True)
                Nt2 = wk_p.tile([128, 128], bf16, tag="Nt2")
                nc.scalar.copy(Nt2[:], ps2[:])
            ps3 = psS.tile([128, 128], f32, tag="sf")
            nc.tensor.matmul(ps3[:], lhsT=N2[:], rhs=Tt[:], start=True, stop=True)
            Tt_new = wk_p.tile([128, 128], bf16, tag="Tt")
            nc.vector.tensor_tensor(Tt_new[:], Tt[:], ps3[:], OP.add)
            Tt = Tt_new
            if i < 6:
                Na, Nta = N2, Nt2
        if _DEBUG and c == 0 and hi == 0:
            nc.sync.dma_start(dbg["N0"][:], N[:])
            nc.sync.dma_start(dbg["Tt0"][:], Tt[:])
        # V chunk: load + transpose to token-major
        Vtm = wk_p.tile([128, DVh], bf16, tag="Vtm")
        for v in range(nvt):
            vt_f = wk_p.tile([128, 128], bf16, tag="vtf")
            nc.sync.dma_start(vt_f[:], vclean_d[(vrow0 + v) * 128:(vrow0 + v + 1) * 128, sl])
            psv = psS.tile([128, 128], bf16, tag="tb")
            nc.tensor.transpose(psv[:], vt_f[:], id_bf_sb[:])
            nc.scalar.copy(Vtm[:, v * 128:(v + 1) * 128], psv[:])
        bV = wk_p.tile([128, DVh], bf16, tag="bV")
        nc.scalar.activation(bV[:], Vtm[:], AF.Copy, bias=0.0, scale=betT[:, j:j + 1])
        # KS0 / QS0
        ps_ks = psB.tile([128, 512], f32, tag="bb")
        nc.tensor.matmul(ps_ks[:, 0:DVh], lhsT=ktA[:, sl], rhs=S[0][:, 0:DVh],
                         start=True, stop=False)
        nc.tensor.matmul(ps_ks[:, 0:DVh], lhsT=ktB[:, sl], rhs=S[1][:, 0:DVh],
                         start=False, stop=True)
        ps_qs = psB.tile([128, 512], f32, tag="bb")
        nc.tensor.matmul(ps_qs[:, 0:DVh], lhsT=qtA[:, sl], rhs=S[0][:, 0:DVh],
                         start=True, stop=False)
        nc.tensor.matmul(ps_qs[:, 0:DVh], lhsT=qtB[:, sl], rhs=S[1][:, 0:DVh],
                         start=False, stop=True)
        qs_sb = wk_p.tile([128, DVh], bf16, tag="qssb")
        nc.scalar.copy(qs_sb[:], ps_qs[:, 0:DVh])
        # R = bV - exp(c1)*KS0
        R = wk_p.tile([128, DVh], bf16, tag="R")
        nc.vector.scalar_tensor_tensor(R[:], in0=ps_ks[:, 0:DVh], scalar=nexp_R[:, j:j + 1],
                                       in1=bV[:], op0=OP.mult, op1=OP.add)
        # U
        ps_u = psB.tile([128, 512], f32, tag="bb")
        nc.tensor.matmul(ps_u[:, 0:DVh], lhsT=Tt[:], rhs=R[:], start=True, stop=True)
        U = wk_p.tile([128, DVh], bf16, tag="U")
        nc.scalar.copy(U[:], ps_u[:, 0:DVh])
        if _DEBUG and c == 0 and hi == 0:
            nc.sync.dma_start(dbg["U0"][:], U[:])
        # O = exp_O * QS0 + At^T U
        ps_au = psB.tile([128, 512], f32, tag="bb")
        nc.tensor.matmul(ps_au[:, 0:DVh], lhsT=At[:], rhs=U[:], start=True, stop=True)
        o_c = wk_p.tile([128, DVh], bf16, tag="oc")
        nc.vector.scalar_tensor_tensor(o_c[:], in0=qs_sb[:], scalar=exp_O[:, j:j + 1],
                                       in1=ps_au[:, 0:DVh], op0=OP.mult, op1=OP.add)
        oscr = wk_p.tile([128, DVh], bf16, tag="oscr")
        nc.scalar.activation(oscr[:], o_c[:], AF.Square, accum_out=ssq_dst[:, c:c + 1])
        nc.sync.dma_start(o_d[c * C:(c + 1) * C, o_off:o_off + DVh], o_c[:])
        # state update
        for d in range(2):
            pst = psS.tile([128, 128], bf16, tag="tb")
            nc.tensor.transpose(pst[:], ktA[:, sl] if d == 0 else ktB[:, sl], id_bf_sb[:])
            ksc = wk_p.tile([128, 128], bf16, tag="ksc")
            nc.scalar.activation(ksc[:], pst[:], AF.Copy, bias=0.0, scale=exp_st[:, j:j + 1])
            ps_s = psB.tile([128, 512], f32, tag="bb")
            nc.tensor.matmul(ps_s[:, 0:DVh], lhsT=ksc[:], rhs=U[:], start=True, stop=True)
            nc.vector.scalar_tensor_tensor(S[d][:, 0:DVh], in0=S[d][:, 0:DVh],
                                           scalar=exp_gc[:, j:j + 1], in1=ps_s[:, 0:DVh],
                                           op0=OP.mult, op1=OP.add)

    for b in range(NOB):
        for cc in range(8):
            c = b * 8 + cc
            chunk_head(c, 0, kt[0], kt[1], qt[0], qt[1], S_A, 512, 0, 0, 4, ssqA)
            chunk_head(c, 1, kt[2], kt[3], qt[2], qt[3], S_B, 256, 512, 4, 2, ssqB)
        # split-head variance allreduce (pairs)
        nc.sync.dma_start(var_in_d[b][:], ssqB[:, b * 8:(b + 1) * 8])
        nc.gpsimd.collective_compute(
            "AllReduce", mybir.AluOpType.add,
            replica_groups=[[0, 1], [2, 3], [4, 5], [6, 7]],
            ins=[var_in_d[b].opt()], outs=[var_out_d[b].opt()])
        vfB = wk_p.tile([128, 8], f32, tag="vfB")
        nc.sync.dma_start(vfB[:], var_out_d[b][:])
        rstdB = wk_p.tile([128, 8], f32, tag="rstdB")
        nc.scalar.activation(rstdB[:], vfB[:], AF.Ln, bias=epsN_sb[:, 0:1], scale=1.0 / DV)
        nc.scalar.activation(rstdB[:], rstdB[:], AF.Exp, scale=-0.5)
        rstdA = wk_p.tile([128, 8], f32, tag="rstdA")
        nc.scalar.activation(rstdA[:], ssqA[:, b * 8:(b + 1) * 8], AF.Ln,
                             bias=epsN_sb[:, 0:1], scale=1.0 / DV)
        nc.scalar.activation(rstdA[:], rstdA[:], AF.Exp, scale=-0.5)
        # per chunk: norm, gate, Wo partial
        for cc in range(8):
            c = b * 8 + cc
            gsb = wk_p.tile([128, HVF], bf16, tag="gate")
            nc.sync.dma_start(gsb[:], gate_d[c * C:(c + 1) * C, :])
            sg = wk_p.tile([128, HVF], f32, tag="sg")
            nc.scalar.activation(sg[:], gsb[:], AF.Exp, scale=-1.0)
            nc.vector.tensor_scalar_add(sg[:], sg[:], 1.0)
            nc.vector.reciprocal(sg[:], sg[:])
            nc.vector.tensor_tensor(gsb[:], gsb[:], sg[:], OP.mult)
            o_ld = wk_p.tile([128, HVF], bf16, tag="old")
            nc.sync.dma_start(o_ld[:], o_d[c * C:(c + 1) * C, :])
            of = wk_p.tile([128, HVF], bf16, tag="of")
            nc.vector.scalar_tensor_tensor(
                of[:, 0:512], in0=o_ld[:, 0:512],
                scalar=rstdA[:, cc:cc + 1], in1=gsb[:, 0:512], op0=OP.mult, op1=OP.mult)
            nc.vector.scalar_tensor_tensor(
                of[:, 512:768], in0=o_ld[:, 512:768],
                scalar=rstdB[:, cc:cc + 1], in1=gsb[:, 512:768], op0=OP.mult, op1=OP.mult)
            oT = wk_p.tile([128, 6 * 128], bf16, tag="oT")
            for i in range(6):
                pso = psS.tile([128, 128], bf16, tag="tb")
                nc.tensor.transpose(pso[:], of[:, i * 128:(i + 1) * 128], id_bf_sb[:])
                nc.scalar.copy(oT[:, i * 128:(i + 1) * 128], pso[:])
            for n in range(4):
                ps = psB.tile([128, 512], f32, tag="bb")
                for i in range(6):
                    nc.tensor.matmul(ps[:], lhsT=oT[:, i * 128:(i + 1) * 128],
                                     rhs=wo_sb[:, i * HID + n * 512: i * HID + (n + 1) * 512],
                                     start=(i == 0), stop=(i == 5))
                osc = wk_p.tile([128, 512], bf16, tag="osc")
                nc.vector.tensor_copy(osc[:], ps[:])
                nc.sync.dma_start(rs_in_d[b][cc * 128:(cc + 1) * 128,
                                            n * 512:(n + 1) * 512], osc[:])
        nc.gpsimd.collective_compute(
            "ReduceScatter", mybir.AluOpType.add,
            replica_groups=[[0, 1, 2, 3], [4, 5, 6, 7]],
            ins=[rs_in_d[b].opt()], outs=[rs_out_d[b].opt()])
        nc.sync.dma_start(out[b * 256:(b + 1) * 256, :], rs_out_d[b][:])

    if _DEBUG:
        nc.sync.dma_start(dbg["o01"][:, 0:HVF], o_d[0:C, :])
        nc.sync.dma_start(dbg["o01"][:, HVF:2 * HVF], o_d[C:2 * C, :])
    es.close()


_CACHED = {}


def _get_program():
    if "nc" not in _CACHED:
        import concourse.bacc as bacc
        nc = bacc.Bacc("TRN2", target_bir_lowering=False)
        _build(nc)
        nc.compile()
        _CACHED["nc"] = nc
    return _CACHED["nc"]


def _wab(Wa, Wb, F, Hh):
    w = np.zeros((HID, 64), np.float32)
    w[:, 0], w[:, 1] = Wa[F], Wa[Hh]
    w[:, 32], w[:, 33] = Wb[F], Wb[Hh]
    return w


def _host_prep(inputs):
    bf = ml_dtypes.bfloat16
    x = np.asarray(inputs["hidden_states"], np.float32)
    Wq, Wk, Wv = (np.asarray(inputs[k], np.float32) for k in ("Wq", "Wk", "Wv"))
    Wb, Wa = np.asarray(inputs["Wb"], np.float32), np.asarray(inputs["Wa"], np.float32)
    A_log = np.asarray(inputs["A_log"], np.float32)
    dt_bias = np.asarray(inputs["dt_bias"], np.float32)
    cwq, cwk, cwv = (np.asarray(inputs[k], np.float32)
                     for k in ("conv_wq", "conv_wk", "conv_wv"))
    Wg = np.asarray(inputs["Wg"], np.float32)
    norm_w = np.asarray(inputs["norm_w"], np.float32)
    Wo = np.asarray(inputs["Wo"], np.float32)

    id128 = np.eye(128, dtype=np.float32)
    m_low = np.tril(np.full((128, 128), -1.0, np.float32), -1)
    m_upp = np.triu(np.full((128, 128), -1.0, np.float32), 1)
    m_upi = np.triu(np.ones((128, 128), np.float32), 0)
    masks = np.ascontiguousarray(np.concatenate([m_low, m_upp, m_upi], axis=1))
    ones = np.ones((128, 128), np.float32)

    in_maps = []
    for core in range(N_CORES):
        b, r = core // 4, core % 4
        F, Hh, lo = FULL_HEAD[r], HALF_HEAD[r], HALF_LO[r]
        qk_rows = np.r_[F * DK:(F + 1) * DK, Hh * DK:(Hh + 1) * DK]
        half0 = Hh * DV + (0 if lo else DV // 2)
        hv_rows = np.r_[F * DV:(F + 1) * DV, half0:half0 + DV // 2]
        nw_idx = np.r_[np.arange(DV), (np.arange(DV // 2) + (0 if lo else DV // 2))]
        im = {
            "xT": np.ascontiguousarray(x[b].T).astype(bf),
            "wqT": np.ascontiguousarray(Wq[qk_rows].T).astype(bf),
            "wkT": np.ascontiguousarray(Wk[qk_rows].T).astype(bf),
            "wabT": _wab(Wa, Wb, F, Hh).astype(bf),
            "wvT": np.ascontiguousarray(Wv[hv_rows].T).astype(bf),
            "wgT": np.ascontiguousarray(Wg[hv_rows].T).astype(bf),
            "woT": np.ascontiguousarray(
                (Wo[:, hv_rows] * norm_w[nw_idx][None, :]).T).astype(bf),
            "convw": np.ascontiguousarray(
                np.concatenate([cwq[qk_rows], cwk[qk_rows], cwv[hv_rows]], 0)),
            "dtb": dt_bias[[F, Hh]].reshape(2, 1).copy(),
            "negA": (-np.exp(A_log[[F, Hh]])).reshape(2, 1).copy(),
            "id_bf": id128.astype(bf),
            "id_f32": id128,
            "ones_bf": ones.astype(bf),
            "masks": masks,
        }
        in_maps.append(im)
    return in_maps


def _assemble(results):
    out = np.empty((2, T, HID), np.float32)
    for core in range(N_CORES):
        b, r = core // 4, core % 4
        sh = np.asarray(results[core]["out"]).astype(np.float32)
        for blk in range(NOB):
            out[b, blk * 1024 + r * 256: blk * 1024 + (r + 1) * 256] = \
                sh[blk * 256:(blk + 1) * 256]
    return out


def _get_runner():
    """Build (once) a cached jitted shard_map runner around the Bass program
    so repeat calls skip jax retracing.  Mirrors bass2jax.run_bass_via_pjrt."""
    if "runner" in _CACHED:
        return _CACHED["runner"]
    import jax
    from jax.sharding import Mesh, PartitionSpec
    from jax.experimental.shard_map import shard_map
    from concourse import bass2jax
    import concourse.mybir as mybir

    nc = _get_program()
    bass2jax.install_neuronx_cc_hook()
    partition_name = nc.partition_id_tensor.name if nc.partition_id_tensor else None
    in_names, out_names, out_avals, zero_shapes = [], [], [], []
    for alloc in nc.m.functions[0].allocations:
        if not isinstance(alloc, mybir.MemoryLocationSet):
            continue
        name = alloc.memorylocations[0].name
        if alloc.kind == "ExternalInput":
            if name != partition_name:
                in_names.append(name)
        elif alloc.kind == "ExternalOutput":
            out_names.append(name)
            shape = tuple(alloc.tensor_shape)
            dtype = mybir.dt.np(alloc.dtype)
            out_avals.append(jax.core.ShapedArray(shape, dtype))
            zero_shapes.append((shape, dtype))
    n_params = len(in_names)
    n_outs = len(out_avals)
    all_in_names = in_names + out_names + ([partition_name] if partition_name else [])

    def _body(*args):
        operands = list(args)
        if partition_name is not None:
            operands.append(bass2jax.partition_id_tensor())
        return tuple(bass2jax._bass_exec_p.bind(
            *operands, out_avals=tuple(out_avals), in_names=tuple(all_in_names),
            out_names=tuple(out_names), lowering_input_output_aliases=(),
            sim_require_finite=True, sim_require_nnan=True, nc=nc))

    devices = jax.devices()[:N_CORES]
    mesh = Mesh(np.asarray(devices), ("core",))
    sharded = jax.jit(
        shard_map(_body, mesh=mesh,
                  in_specs=(PartitionSpec("core"),) * (n_params + n_outs),
                  out_specs=(PartitionSpec("core"),) * n_outs,
                  check_rep=False),
        donate_argnums=tuple(range(n_params, n_params + n_outs)),
        keep_unused=True)

    def run(in_maps):
        per_core = [[np.asarray(m[name]) for name in in_names] for m in in_maps]
        concat_in = [np.concatenate([per_core[c][i] for c in range(N_CORES)], axis=0)
                     for i in range(n_params)]
        concat_zeros = [np.zeros((N_CORES * s[0], *s[1:]), d) for s, d in zero_shapes]
        outs = sharded(*concat_in, *concat_zeros)
        return [
            {name: np.asarray(outs[i]).reshape(N_CORES, *out_avals[i].shape)[c]
             for i, name in enumerate(out_names)}
            for c in range(N_CORES)
        ]

    _CACHED["runner"] = run
    _CACHED["runner_parts"] = (sharded, in_names, out_names, out_avals, zero_shapes, mesh)
    return run


def bench_exec(inputs, iters=3):
    """Steady-state execute-only wall times (inputs pre-staged on device)."""
    import jax, time
    from jax.sharding import NamedSharding, PartitionSpec
    run = _get_runner()
    sharded, in_names, out_names, out_avals, zero_shapes, mesh = _CACHED["runner_parts"]
    in_maps = _host_prep(inputs)
    per_core = [[np.asarray(m[name]) for name in in_names] for m in in_maps]
    concat_in = [np.concatenate([per_core[c][i] for c in range(N_CORES)], axis=0)
                 for i in range(len(in_names))]
    sh = NamedSharding(mesh, PartitionSpec("core"))
    dev_in = [jax.device_put(x, sh) for x in concat_in]
    jax.block_until_ready(dev_in)
    times = []
    outs = None
    for _ in range(iters):
        concat_zeros = [jax.device_put(np.zeros((N_CORES * s[0], *s[1:]), d), sh)
                        for s, d in zero_shapes]
        jax.block_until_ready(concat_zeros)
        t0 = time.time()
        outs = sharded(*dev_in, *concat_zeros)
        jax.block_until_ready(outs)
        times.append(time.time() - t0)
    results = [
        {name: np.asarray(outs[i]).reshape(N_CORES, *out_avals[i].shape)[c]
         for i, name in enumerate(out_names)}
        for c in range(N_CORES)
    ]
    return times, _assemble(results)


def run_device(inputs, trace=False, trace_kwargs=None):
    in_maps = _host_prep(inputs)
    if not trace:
        results = _get_runner()(in_maps)
        return _assemble(results), results
    from concourse.bass_utils import run_bass_kernel_spmd
    nc = _get_program()
    kw = {"trace": True}
    if trace_kwargs:
        kw["trace_kwargs"] = trace_kwargs
    res = run_bass_kernel_spmd(nc, in_maps, list(range(N_CORES)), **kw)
    return _assemble(res.results), res


def kernel(**inputs):
    try:
        out, _ = run_device(inputs)
        return out
    except Exception as e:
        import traceback
        traceback.print_exc()
        print("DEVICE PATH FAILED - falling back to host numpy:", repr(e))
        return _host_fallback(**inputs)


# ---------------- host fallback (correct but slow) ----------------
def _silu(x):
    return x * (1.0 / (1.0 + np.exp(-x)))


def _host_fallback(hidden_states, Wq, Wk, Wv, Wb, Wa, A_log, dt_bias,
                   conv_wq, conv_wk, conv_wv, Wg, norm_w, Wo):
    x = np.asarray(hidden_states, np.float32)
    B = x.shape[0]

    def conv(u, w):
        y = u * w[None, None, :, CONV - 1]
        for j in range(CONV - 1):
            s = CONV - 1 - j
            y[:, s:, :] += u[:, :-s, :] * w[None, None, :, j]
        return y

    xf = x.reshape(B * T, HID)
    q = _silu(conv((xf @ Wq.T).reshape(B, T, H * DK), conv_wq)).reshape(B, T, H, DK)
    k = _silu(conv((xf @ Wk.T).reshape(B, T, H * DK), conv_wk)).reshape(B, T, H, DK)
    v = _silu(conv((xf @ Wv.T).reshape(B, T, H * DV), conv_wv)).reshape(B, T, H, DV)
    beta = 1.0 / (1.0 + np.exp(-(xf @ Wb.T))).reshape(B, T, H)
    g = (-np.exp(A_log)[None, :] *
         np.logaddexp(0.0, xf @ Wa.T + dt_bias)).reshape(B, T, H)
    q = q / np.sqrt((q * q).sum(-1, keepdims=True) + 1e-6) * DK ** -0.5
    k = k / np.sqrt((k * k).sum(-1, keepdims=True) + 1e-6)
    BHn = B * H
    qt = np.moveaxis(q, (0, 2), (1, 2)).reshape(T, BHn, DK)
    kt = np.moveaxis(k, (0, 2), (1, 2)).reshape(T, BHn, DK)
    vt = np.moveaxis(v, (0, 2), (1, 2)).reshape(T, BHn, DV)
    eg = np.exp(np.moveaxis(g, (0, 2), (1, 2)).reshape(T, BHn))
    bt = np.moveaxis(beta, (0, 2), (1, 2)).reshape(T, BHn)
    S = np.zeros((BHn, DK, DV), np.float32)
    o = np.empty((T, BHn, DV), np.float32)
    for t in range(T):
        S *= eg[t][:, None, None]
        v_new = vt[t] - np.einsum('bk,bkv->bv', kt[t], S, optimize=True)
        S += (bt[t][:, None] * kt[t])[:, :, None] * v_new[:, None, :]
        o[t] = np.einsum('bk,bkv->bv', qt[t], S, optimize=True)
    o = np.moveaxis(o.reshape(T, B, H, DV), 0, 1)
    var = np.mean(o * o, axis=-1, keepdims=True)
    o = o / np.sqrt(var + NORM_EPS) * norm_w
    gate = (xf @ Wg.T).reshape(B, T, H, DV)
    o = o * _silu(gate)
    return (o.reshape(B, T, H * DV) @ Wo.T).astype(np.float32)
